# revision 1
# baseline (speedup 1.0000x reference)
"""ComplexMultiHeadAttention on 8 TRN2 NeuronCores (Bass/Tile).

Problem: B=4, S=1024, D_MODEL=1024, N_HEADS=16, D_HEAD=64, complex-valued
activations stored as a trailing dim of size 2 (real, imag).

    q = to_heads(complex_linear(queries, wq));  k, v likewise
    s_r + i*s_i = (q_r + i q_i)(k_r + i k_i)^T / sqrt(dh)
    a_r = softmax(s_r), a_i = softmax(s_i)      (independent softmaxes)
    o = complex_bmm(a, v);  out = complex_linear(concat_heads(o), wo)

Sharding: head-parallel. Core c owns heads {2c, 2c+1} = 128 contiguous dims
of the hidden axis. Each core computes Q/K/V projections for its 128 output
dims (weights row-sliced), runs attention for its 8 (batch, head) pairs, and
computes a partial O-projection (wo column-sliced on its 128 input dims)
over all 1024 output dims. The host sums the 8 partial outputs — no
on-device collectives.

Performance notes (TRN2):
  - The PE p-state ramps: any idle gap drops the clock to 1.2 GHz for the
    next ~3us. The whole program is therefore emitted as one continuous
    tensor stream: attention(b) -> proj(b+1) -> oproj(b) -> attention(b+1),
    with the attention inner loop software-pipelined (scores for chunk n+1
    are emitted before the exp-gated consumers of chunk n).
  - DMA cost is per-partition-LINE (~5.6ns/line regardless of 1KB vs 2KB),
    so inputs use a partition-major DRAM layout ([gt*128+p, dc*512+tok])
    giving 8KB contiguous lines: one 1MB DMA per (tensor, part, gt) instead
    of eight 128KB DMAs with 1KB lines. Outputs are batched the same way.
  - V is transposed to token-major via DMA-transpose on the scalar HWDGE
    queue (not the PE), with V projected FIRST so the transposes overlap
    the q/k projection matmuls.
  - scores r|i land in the two banks of one wide [128,1024] PSUM tile; a
    single wide EXP covers both (scalar engine issue rate ~1.15us/unit vs
    1.28us of tensor work per unit -> attention stays tensor-paced).
  - All matmuls bf16 (f32 PSUM accumulation); softmax over k (=partitions)
    skips max-subtraction (scores are O(1) by construction) and takes Z
    with a ones[128,128] matmul; 1/Z is one wide reciprocal + one wide mul.
PSUM budget (8 banks): wide proj/score pool 2x2 + wide Z 2 + wide AV 2.
"""

import os
import numpy as np
import ml_dtypes
from contextlib import ExitStack

import concourse.bass as bass
import concourse.tile as tile
from concourse import bacc, mybir

F32 = mybir.dt.float32
BF16 = mybir.dt.bfloat16
EXP = mybir.ActivationFunctionType.Exp

B, S, D, H, DH = 4, 1024, 1024, 16, 64
NCORES = 8
P = 128            # partitions / chunk size
TBLK = 512         # token block (matmul free dim)
WBLK = 2 * TBLK    # wide psum tile (2 banks)
DC = D // P        # 8 d-chunks
KC = S // P        # 8 key chunks per batch
HPC = H // NCORES  # 2 heads per core
NT = (B * S) // TBLK

_CACHE = {}


def _build():
    nc = bacc.Bacc("TRN2", target_bir_lowering=False, debug=False,
                   num_devices=NCORES)

    # partition-major tiled layout: row gt*128+p, col dc*512+tok
    x_ap = {}
    for t in ("q", "k", "v"):
        for part in ("r", "i"):
            x_ap[t + part] = nc.dram_tensor(
                f"x{t}_{part}", [NT * P, DC * TBLK],
                BF16, kind="ExternalInput").ap()
    w_ap = {}
    for t in ("q", "k", "v"):
        for h in range(HPC):
            for suf in ("a", "b"):
                w_ap[f"{t}{suf}{h}"] = nc.dram_tensor(
                    f"w{t}_{suf}{h}", [P, D], BF16, kind="ExternalInput").ap()
    wo_ap = {}
    for suf in ("r", "i", "in"):
        wo_ap[suf] = nc.dram_tensor(
            f"wo_{suf}", [P, D], BF16, kind="ExternalInput").ap()
    ones_ap = nc.dram_tensor("onesin", [P, P], BF16, kind="ExternalInput").ap()
    # output: row gt*128+p, col (2*mc+ri)*512+tok  (r/i interleaved per mc)
    po_ap = nc.dram_tensor("po", [NT * P, 2 * DC * TBLK], BF16,
                           kind="ExternalOutput").ap()

    with tile.TileContext(nc) as tc, ExitStack() as ctx:
        wpool = ctx.enter_context(tc.tile_pool(name="w", bufs=1))
        xpool = ctx.enter_context(tc.tile_pool(name="x", bufs=7))
        qkpool = ctx.enter_context(tc.tile_pool(name="qk", bufs=2))
        vpool = ctx.enter_context(tc.tile_pool(name="v", bufs=2))
        opool = ctx.enter_context(tc.tile_pool(name="ost", bufs=2))
        upool = ctx.enter_context(tc.tile_pool(name="u", bufs=3))
        zpool = ctx.enter_context(tc.tile_pool(name="z", bufs=2))
        tmppool = ctx.enter_context(tc.tile_pool(name="tmp", bufs=2))
        popool = ctx.enter_context(tc.tile_pool(name="po", bufs=2))
        vstpool = ctx.enter_context(tc.tile_pool(name="vst", bufs=4))
        # PSUM: 8 banks: wide 2-bank [128,1024] proj/score/o-proj pool x2,
        # plus FOUR separate 1-bank accumulators (zr, zi, oa, ob) so each
        # frees as soon as its own epilogue read completes (the next octet's
        # kc0 matmuls reuse them ~1.4us after the previous octet ends).
        sps = ctx.enter_context(tc.tile_pool(name="sp", bufs=2, space="PSUM"))
        zps_pool = ctx.enter_context(tc.tile_pool(name="zp", bufs=1,
                                                  space="PSUM"))
        ops_pool = ctx.enter_context(tc.tile_pool(name="op", bufs=1,
                                                  space="PSUM"))

        # weights on the scalar HWDGE queue (idle at start; sync queue
        # starts on the batch-0 x loads in parallel)
        wt = {}
        for key, ap in list(w_ap.items()):
            wt[key] = wpool.tile([P, D], BF16, tag=f"w_{key}", name=f"w_{key}")
            nc.scalar.dma_start(wt[key][:], ap[:])
        wot = {}
        for suf, ap in wo_ap.items():
            wot[suf] = wpool.tile([P, D], BF16, tag=f"wo_{suf}",
                                  name=f"wo_{suf}")
            nc.scalar.dma_start(wot[suf][:], ap[:])
        ones = wpool.tile([P, P], BF16, tag="ones", name="ones")
        nc.scalar.dma_start(ones[:], ones_ap[:])

        xtiles = {}

        def emit_xloads(b):
            # v first (projection order is v,q,k)
            for t in ("v", "q", "k"):
                for part in ("r", "i"):
                    for half in range(2):
                        gt = 2 * b + half
                        xt = xpool.tile([P, DC * TBLK], BF16, tag="xt",
                                        name="xt")
                        nc.sync.dma_start(
                            xt[:], x_ap[t + part][gt * P:(gt + 1) * P, :])
                        xtiles[(b, t, part, half)] = xt

        def emit_proj(b, qcat, kcr, kci, va, vb):
            # v FIRST so its DMA-transposes (scalar queue) overlap the q/k
            # projection matmuls
            for t in ("v", "q", "k"):
                wA = (wt[t + "a0"], wt[t + "a1"])
                wB = (wt[t + "b0"], wt[t + "b1"])
                for half in range(2):
                    xr = xtiles.pop((b, t, "r", half))
                    xi = xtiles.pop((b, t, "i", half))
                    pwide = sps.tile([P, WBLK], F32, tag="sps", name="projw")
                    psr = pwide[:, 0:TBLK]
                    psi = pwide[:, TBLK:WBLK]
                    for dc in range(DC):
                        ws = slice(dc * P, (dc + 1) * P)
                        xs_ = slice(dc * TBLK, (dc + 1) * TBLK)
                        nc.tensor.matmul(psr, wA[0][:, ws], xr[:, xs_],
                                         start=(dc == 0), stop=False)
                        nc.tensor.matmul(psi, wA[1][:, ws], xr[:, xs_],
                                         start=(dc == 0), stop=False)
                    for dc in range(DC):
                        ws = slice(dc * P, (dc + 1) * P)
                        xs_ = slice(dc * TBLK, (dc + 1) * TBLK)
                        nc.tensor.matmul(psr, wB[0][:, ws], xi[:, xs_],
                                         start=False, stop=(dc == DC - 1))
                        nc.tensor.matmul(psi, wB[1][:, ws], xi[:, xs_],
                                         start=False, stop=(dc == DC - 1))
                    hs = slice(half * TBLK, (half + 1) * TBLK)
                    if t == "q":
                        # psX = [q_r(h); q_i(h)] = Qcat directly
                        for h, psx in ((0, psr), (1, psi)):
                            nc.vector.tensor_copy(qcat[h][:, hs], psx)
                    elif t == "k":
                        # psX = [k_r(h); -k_i(h)] = Kcat_r directly;
                        # Kcat_i = [k_i; k_r] via one negate + one copy
                        for h, psx in ((0, psr), (1, psi)):
                            nc.vector.tensor_copy(kcr[h][:, hs], psx)
                            nc.vector.tensor_scalar_mul(
                                kci[h][0:DH, hs], psx[DH:P, :], -1.0)
                            nc.vector.tensor_copy(kci[h][DH:P, hs],
                                                  psx[0:DH, :])
                    else:
                        # psr = [v_r(h0); v_i(h0)], psi = [v_r(h1); v_i(h1)]
                        # DMA-transpose (scalar HWDGE) to token-major packs
                        for h, psx in ((0, psr), (1, psi)):
                            vst = vstpool.tile([P, TBLK], BF16, tag="vst",
                                               name="vst")
                            nc.vector.tensor_copy(vst[:], psx)
                            for blk in range(4):
                                kcg = half * 4 + blk
                                nc.scalar.dma_start(
                                    va[h][:, kcg * P:(kcg + 1) * P],
                                    vst[:, blk * P:(blk + 1) * P],
                                    transpose=True)
                            # vb = [v_i | v_r] per key chunk (the minus of
                            # the complex product sits in the epilogue sub)
                            base = half * TBLK
                            vbv = vb[h][:, base:base + TBLK].rearrange(
                                "p (k c) -> p k c", c=P)
                            vav = va[h][:, base:base + TBLK].rearrange(
                                "p (k c) -> p k c", c=P)
                            nc.vector.tensor_copy(vbv[:, :, 0:DH],
                                                  vav[:, :, DH:P])
                            nc.vector.tensor_copy(vbv[:, :, DH:P],
                                                  vav[:, :, 0:DH])

        def emit_attention(b, qcat, kcr, kci, va, vb, o_stage):
            units = [(h, qb, kc)
                     for h in range(HPC) for qb in range(2)
                     for kc in range(KC)]
            swides = [None] * len(units)
            accs = {}

            def emit_scores(n):
                h, qb, kc = units[n]
                qs = slice(qb * TBLK, (qb + 1) * TBLK)
                ks = slice(kc * P, (kc + 1) * P)
                sw = sps.tile([P, WBLK], F32, tag="sps", name="scorew")
                nc.tensor.matmul(sw[:, 0:TBLK], kcr[h][:, ks],
                                 qcat[h][:, qs], start=True, stop=True)
                nc.tensor.matmul(sw[:, TBLK:WBLK], kci[h][:, ks],
                                 qcat[h][:, qs], start=True, stop=True)
                swides[n] = sw

            emit_scores(0)
            for n, (h, qb, kc) in enumerate(units):
                if n + 1 < len(units):
                    emit_scores(n + 1)
                uw = upool.tile([P, WBLK], BF16, tag="u", name="u")
                nc.scalar.activation(uw[:], swides[n][:], EXP)
                ur = uw[:, 0:TBLK]
                ui = uw[:, TBLK:WBLK]
                ks = slice(kc * P, (kc + 1) * P)
                first, last = kc == 0, kc == KC - 1
                if first:
                    zr = zps_pool.tile([P, TBLK], F32, tag="zr", name="zr")
                    zi = zps_pool.tile([P, TBLK], F32, tag="zi", name="zi")
                    oa = ops_pool.tile([P, TBLK], F32, tag="oa", name="oa")
                    ob = ops_pool.tile([P, TBLK], F32, tag="ob", name="ob")
                    accs[(h, qb)] = (zr, zi, oa, ob)
                else:
                    zr, zi, oa, ob = accs[(h, qb)]
                nc.tensor.matmul(zr[:], ones[:], ur,
                                 start=first, stop=last)
                nc.tensor.matmul(zi[:], ones[:], ui,
                                 start=first, stop=last)
                nc.tensor.matmul(oa[:], va[h][:, ks], ur,
                                 start=first, stop=last)
                nc.tensor.matmul(ob[:], vb[h][:, ks], ui,
                                 start=first, stop=last)
                if last:
                    # o_r = (v_r.T u_r)/Z_r - (v_i.T u_i)/Z_i : each AV term
                    # gets its OWN softmax denominator.  Ordered so each
                    # accumulator bank frees as early as possible.
                    qs = slice(qb * TBLK, (qb + 1) * TBLK)
                    zinv_r = zpool.tile([P, TBLK], F32, tag="zinvr",
                                        name="zinv_r")
                    nc.vector.reciprocal_approx_fast(zinv_r[:], zr[:])
                    zinv_i = zpool.tile([P, TBLK], F32, tag="zinvi",
                                        name="zinv_i")
                    nc.vector.reciprocal_approx_fast(zinv_i[:], zi[:])
                    tmpa = tmppool.tile([P, TBLK], F32, tag="tmpa",
                                        name="tmpa")
                    nc.vector.tensor_mul(tmpa[:], oa[:], zinv_r[:])
                    tmpb = tmppool.tile([P, TBLK], F32, tag="tmpb",
                                        name="tmpb")
                    nc.vector.tensor_mul(tmpb[:], ob[:], zinv_i[:])
                    dst = slice(DH * h, DH * (h + 1))
                    nc.vector.tensor_sub(o_stage["r"][dst, qs],
                                         tmpa[0:DH, :], tmpb[0:DH, :])
                    nc.vector.tensor_add(o_stage["i"][dst, qs],
                                         tmpa[DH:P, :], tmpb[DH:P, :])

        def emit_oproj(b, o_stage):
            for half in range(2):
                hs = slice(half * TBLK, (half + 1) * TBLK)
                gt = 2 * b + half
                powide = popool.tile([P, 2 * DC * TBLK], BF16, tag="pow",
                                     name="powide")
                for mc in range(DC):
                    ms = slice(mc * P, (mc + 1) * P)
                    ow2 = sps.tile([P, WBLK], F32, tag="sps", name="ojw")
                    pr = ow2[:, 0:TBLK]
                    pi = ow2[:, TBLK:WBLK]
                    nc.tensor.matmul(pr, wot["r"][:, ms],
                                     o_stage["r"][:, hs],
                                     start=True, stop=False)
                    nc.tensor.matmul(pr, wot["in"][:, ms],
                                     o_stage["i"][:, hs],
                                     start=False, stop=True)
                    nc.tensor.matmul(pi, wot["i"][:, ms],
                                     o_stage["r"][:, hs],
                                     start=True, stop=False)
                    nc.tensor.matmul(pi, wot["r"][:, ms],
                                     o_stage["i"][:, hs],
                                     start=False, stop=True)
                    # one wide copy: [pr | pi] -> powide cols 2*mc..2*mc+2
                    nc.vector.tensor_copy(
                        powide[:, 2 * mc * TBLK:(2 * mc + 2) * TBLK], ow2[:])
                # store on the gpsimd SWDGE queue: the sync queue must stay
                # dedicated to input prefetch (a store blocking the sync
                # FIFO head starves the next projection phase)
                nc.gpsimd.dma_start(po_ap[gt * P:(gt + 1) * P, :], powide[:])

        # ---- pipelined emission: one continuous tensor stream ----
        emit_xloads(0)
        stage = {}

        def new_stage(b):
            qcat = [qkpool.tile([P, S], BF16, tag=f"qcat{h}", name=f"qcat{h}")
                    for h in range(HPC)]
            kcr = [qkpool.tile([P, S], BF16, tag=f"kcr{h}", name=f"kcr{h}")
                   for h in range(HPC)]
            kci = [qkpool.tile([P, S], BF16, tag=f"kci{h}", name=f"kci{h}")
                   for h in range(HPC)]
            va = [vpool.tile([P, S], BF16, tag=f"va{h}", name=f"va{h}")
                  for h in range(HPC)]
            vb = [vpool.tile([P, S], BF16, tag=f"vb{h}", name=f"vb{h}")
                  for h in range(HPC)]
            o_stage = {p: opool.tile([P, S], BF16, tag=f"ost{p}",
                                     name=f"ost{p}")
                       for p in ("r", "i")}
            stage[b] = (qcat, kcr, kci, va, vb, o_stage)

        new_stage(0)
        emit_proj(0, *stage[0][:5])
        for b in range(B):
            if b + 1 < B:
                emit_xloads(b + 1)
            emit_attention(b, *stage[b])
            if b + 1 < B:
                new_stage(b + 1)
                emit_proj(b + 1, *stage[b + 1][:5])
            emit_oproj(b, stage[b][5])
            del stage[b]

    nc.compile()
    return nc


def _w_sbuf_layout(w_t):
    """[D, 128] weight-transpose slice -> SBUF layout [128, dc*128+o]."""
    return np.ascontiguousarray(
        w_t.reshape(DC, P, P).transpose(1, 0, 2).reshape(P, D))


def _tile_x(xT, dtype):
    """[D, B*S] -> partition-major [NT*P, DC*TBLK] (row gt*P+p, col dc*TBLK+t)."""
    t = xT.reshape(DC, P, NT, TBLK).transpose(2, 1, 0, 3)
    return np.ascontiguousarray(t.reshape(NT * P, DC * TBLK)).astype(dtype)


def _prepare_in_maps(inputs):
    bf = ml_dtypes.bfloat16
    xs = {}
    for name, t in (("queries", "q"), ("keys", "k"), ("values", "v")):
        x = np.asarray(inputs[name], dtype=np.float32)  # [B,S,D,2]
        flat = x.reshape(B * S, D, 2)
        xs[t + "r"] = _tile_x(flat[:, :, 0].T, bf)
        xs[t + "i"] = _tile_x(flat[:, :, 1].T, bf)

    scale = np.float32(1.0 / np.sqrt(DH))
    in_maps = []
    for c in range(NCORES):
        rows = slice(P * c, P * (c + 1))
        m = {}
        for t in ("q", "k", "v"):
            for part in ("r", "i"):
                m[f"x{t}_{part}"] = xs[t + part]
        for t, wr_name, wi_name in (("q", "wq_r", "wq_i"),
                                    ("k", "wk_r", "wk_i"),
                                    ("v", "wv_r", "wv_i")):
            s = scale if t == "q" else np.float32(1.0)
            wr = np.asarray(inputs[wr_name], dtype=np.float32)[rows] * s
            wi = np.asarray(inputs[wi_name], dtype=np.float32)[rows] * s
            for h in range(HPC):
                hr = slice(DH * h, DH * (h + 1))
                if t == "q":
                    wa = np.concatenate([wr[hr].T, wi[hr].T], axis=1)
                    wb = np.concatenate([-wi[hr].T, wr[hr].T], axis=1)
                elif t == "k":
                    wa = np.concatenate([wr[hr].T, -wi[hr].T], axis=1)
                    wb = np.concatenate([-wi[hr].T, -wr[hr].T], axis=1)
                else:
                    wa = np.concatenate([wr[hr].T, wi[hr].T], axis=1)
                    wb = np.concatenate([-wi[hr].T, wr[hr].T], axis=1)
                m[f"w{t}_a{h}"] = _w_sbuf_layout(wa).astype(bf)
                m[f"w{t}_b{h}"] = _w_sbuf_layout(wb).astype(bf)
        wo_r = np.asarray(inputs["wo_r"], dtype=np.float32)[:, rows]  # [D,128]
        wo_i = np.asarray(inputs["wo_i"], dtype=np.float32)[:, rows]
        m["wo_r"] = np.ascontiguousarray(wo_r.T).astype(bf)  # [128 d, 1024 m]
        m["wo_i"] = np.ascontiguousarray(wo_i.T).astype(bf)
        m["wo_in"] = np.ascontiguousarray(-wo_i.T).astype(bf)
        m["onesin"] = np.ones((P, P), dtype=bf)
        in_maps.append(m)
    return in_maps


LAST_RESULT = None


def _run(inputs, trace=False):
    global LAST_RESULT
    from concourse.bass_utils import run_bass_kernel_spmd
    if "nc" not in _CACHE:
        _CACHE["nc"] = _build()
    nc = _CACHE["nc"]
    in_maps = _prepare_in_maps(inputs)
    if trace:
        os.environ.pop("BASS_NEVER_TRACE", None)
    else:
        os.environ["BASS_NEVER_TRACE"] = "1"
    res = run_bass_kernel_spmd(nc, in_maps, core_ids=list(range(NCORES)),
                               trace=trace)
    LAST_RESULT = res
    # po rows gt*P+p, cols (2*mc+ri)*TBLK+tok
    acc = np.zeros((NT * P, 2 * DC * TBLK), np.float32)
    for c in range(NCORES):
        acc += res.results[c]["po"].astype(np.float32)

    # [NT, P, DC, 2, TBLK] -> ri, [D=DC*P? no: d=mc*P+p] , tokens
    t = acc.reshape(NT, P, DC, 2, TBLK)
    out = np.empty((B, S, D, 2), np.float32)
    for ri in range(2):
        # value at [gt, p, mc, ri, tok] = out_part[d=mc*128+p, gt*512+tok]
        comp = t[:, :, :, ri, :].transpose(2, 1, 0, 3).reshape(D, B * S)
        out[..., ri] = comp.T.reshape(B, S, D)
    return out


def kernel(**inputs):
    return _run(inputs, trace=False)



# revision 10
# speedup vs baseline: 1.1917x; 1.1917x over previous
"""ComplexMultiHeadAttention on 8 TRN2 NeuronCores (Bass/Tile).

Problem: B=4, S=1024, D_MODEL=1024, N_HEADS=16, D_HEAD=64, complex-valued
activations stored as a trailing dim of size 2 (real, imag).

    q = to_heads(complex_linear(queries, wq));  k, v likewise
    s_r + i*s_i = (q_r + i q_i)(k_r + i k_i)^T / sqrt(dh)
    a_r = softmax(s_r), a_i = softmax(s_i)      (independent softmaxes)
    o = complex_bmm(a, v);  out = complex_linear(concat_heads(o), wo)

Sharding: head-parallel. Core c owns heads {2c, 2c+1} = 128 contiguous dims
of the hidden axis. Each core computes Q/K/V projections for its 128 output
dims (weights row-sliced), runs attention for its 8 (batch, head) pairs, and
computes a partial O-projection (wo column-sliced on its 128 input dims)
over all 1024 output dims. The host sums the 8 partial outputs — no
on-device collectives.

Performance notes (TRN2):
  - The PE p-state ramps: any idle gap drops the clock to 1.2 GHz for the
    next ~3us. The whole program is therefore emitted as one continuous
    tensor stream: attention(b) -> proj(b+1) -> oproj(b) -> attention(b+1),
    with the attention inner loop software-pipelined (scores for chunk n+1
    are emitted before the exp-gated consumers of chunk n).
  - DMA cost is per-partition-LINE (~5.6ns/line regardless of 1KB vs 2KB),
    so inputs use a partition-major DRAM layout ([gt*128+p, dc*512+tok])
    giving 8KB contiguous lines: one 1MB DMA per (tensor, part, gt) instead
    of eight 128KB DMAs with 1KB lines. Outputs are batched the same way.
  - V is transposed to token-major via DMA-transpose on the scalar HWDGE
    queue (not the PE), with V projected FIRST so the transposes overlap
    the q/k projection matmuls.
  - scores r|i land in the two banks of one wide [128,1024] PSUM tile; a
    single wide EXP covers both (scalar engine issue rate ~1.15us/unit vs
    1.28us of tensor work per unit -> attention stays tensor-paced).
  - All matmuls bf16 (f32 PSUM accumulation); softmax over k (=partitions)
    skips max-subtraction (scores are O(1) by construction) and takes Z
    with a ones[128,128] matmul; 1/Z is one wide reciprocal + one wide mul.
PSUM budget (8 banks): wide proj/score pool 2x2 + wide Z 2 + wide AV 2.
"""

import os
import numpy as np
import ml_dtypes
from contextlib import ExitStack

import concourse.bass as bass
import concourse.tile as tile
from concourse import bacc, mybir

F32 = mybir.dt.float32
BF16 = mybir.dt.bfloat16
EXP = mybir.ActivationFunctionType.Exp

B, S, D, H, DH = 4, 1024, 1024, 16, 64
NCORES = 8
P = 128            # partitions / chunk size
TBLK = 512         # token block (matmul free dim)
WBLK = 2 * TBLK    # wide psum tile (2 banks)
DC = D // P        # 8 d-chunks
KC = S // P        # 8 key chunks per batch
HPC = H // NCORES  # 2 heads per core
NT = (B * S) // TBLK

_CACHE = {}


def _build():
    nc = bacc.Bacc("TRN2", target_bir_lowering=False, debug=False,
                   num_devices=NCORES)

    # partition-major tiled layout: row gt*128+p, col dc*512+tok
    x_ap = {}
    for t in ("q", "k", "v"):
        for part in ("r", "i"):
            x_ap[t + part] = nc.dram_tensor(
                f"x{t}_{part}", [NT * P, DC * TBLK],
                BF16, kind="ExternalInput").ap()
    w_ap = {}
    for t in ("q", "k"):
        for h in range(HPC):
            for suf in ("a", "b"):
                w_ap[f"{t}{suf}{h}"] = nc.dram_tensor(
                    f"w{t}_{suf}{h}", [P, D], BF16, kind="ExternalInput").ap()
    # V^T-projection weights: [128 d-in-chunk, dc*256 + (h*128 + comp)]
    wvt_ap = {}
    for part in ("r", "i"):
        wvt_ap[part] = nc.dram_tensor(
            f"wvt_{part}", [P, DC * 2 * P], BF16, kind="ExternalInput").ap()
    wo_ap = {}
    for suf in ("r", "i", "in"):
        wo_ap[suf] = nc.dram_tensor(
            f"wo_{suf}", [P, D], BF16, kind="ExternalInput").ap()
    ones_ap = nc.dram_tensor("onesin", [P, P], BF16, kind="ExternalInput").ap()
    # output: row gt*128+p, col (2*mc+ri)*512+tok  (r/i interleaved per mc)
    po_ap = nc.dram_tensor("po", [NT * P, 2 * DC * TBLK], BF16,
                           kind="ExternalOutput").ap()

    with tile.TileContext(nc) as tc, ExitStack() as ctx:
        wpool = ctx.enter_context(tc.tile_pool(name="w", bufs=1))
        xpool = ctx.enter_context(tc.tile_pool(name="x", bufs=7))
        qkpool = ctx.enter_context(tc.tile_pool(name="qk", bufs=2))
        vpool = ctx.enter_context(tc.tile_pool(name="v", bufs=2))
        opool = ctx.enter_context(tc.tile_pool(name="ost", bufs=2))
        upool = ctx.enter_context(tc.tile_pool(name="u", bufs=3))
        zpool = ctx.enter_context(tc.tile_pool(name="z", bufs=2))
        tmppool = ctx.enter_context(tc.tile_pool(name="tmp", bufs=2))
        popool = ctx.enter_context(tc.tile_pool(name="po", bufs=2))
        # PSUM: 8 banks: wide 2-bank [128,1024] proj/score/o-proj pool x2,
        # plus FOUR separate 1-bank accumulators (zr, zi, oa, ob) so each
        # frees as soon as its own epilogue read completes (the next octet's
        # kc0 matmuls reuse them ~1.4us after the previous octet ends).
        sps = ctx.enter_context(tc.tile_pool(name="sp", bufs=2, space="PSUM"))
        zps_pool = ctx.enter_context(tc.tile_pool(name="zp", bufs=1,
                                                  space="PSUM"))
        ops_pool = ctx.enter_context(tc.tile_pool(name="op", bufs=1,
                                                  space="PSUM"))

        # weights on the scalar HWDGE queue (idle at start; sync queue
        # starts on the batch-0 x loads in parallel).  V^T weights first:
        # they gate the very first projection matmuls.
        wvt = {}
        for part in ("r", "i"):
            wvt[part] = wpool.tile([P, DC * 2 * P], BF16, tag=f"wvt_{part}",
                                   name=f"wvt_{part}")
            nc.scalar.dma_start(wvt[part][:], wvt_ap[part][:])
        wt = {}
        for key, ap in list(w_ap.items()):
            wt[key] = wpool.tile([P, D], BF16, tag=f"w_{key}", name=f"w_{key}")
            nc.scalar.dma_start(wt[key][:], ap[:])
        wot = {}
        for suf, ap in wo_ap.items():
            wot[suf] = wpool.tile([P, D], BF16, tag=f"wo_{suf}",
                                  name=f"wo_{suf}")
            nc.scalar.dma_start(wot[suf][:], ap[:])
        ones = wpool.tile([P, P], BF16, tag="ones", name="ones")
        nc.scalar.dma_start(ones[:], ones_ap[:])

        xtiles = {}

        def emit_xloads(b):
            # v first (projection order is v,q,k); both parts of a half
            # together (the V^T projection consumes r+i of one half first)
            for t in ("v", "q", "k"):
                for half in range(2):
                    for part in ("r", "i"):
                        gt = 2 * b + half
                        xt = xpool.tile([P, DC * TBLK], BF16, tag="xt",
                                        name="xt")
                        nc.sync.dma_start(
                            xt[:], x_ap[t + part][gt * P:(gt + 1) * P, :])
                        xtiles[(b, t, part, half)] = xt

        def emit_proj(b, qcat, kcr, kci, va):
            # v FIRST: attention unit 0 reads va; V^T lands token-major
            # straight out of the PE (no DMA transposes).
            for t in ("v", "q", "k"):
                for half in range(2):
                    xr = xtiles.pop((b, t, "r", half))
                    xi = xtiles.pop((b, t, "i", half))
                    if t == "v":
                        # V^T projection: psum [128 tokens, 256 comps]
                        # (cols h*128 + [v_r(64)|v_i(64)]), accumulated over
                        # d-chunks with X slices stationary, weights moving.
                        for tb in range(4):
                            kc = half * 4 + tb
                            vps = sps.tile([P, 2 * P], F32, tag="sps",
                                           name="vps")
                            for dc in range(DC):
                                xs_ = slice(dc * TBLK + tb * P,
                                            dc * TBLK + (tb + 1) * P)
                                ws = slice(dc * 2 * P, (dc + 1) * 2 * P)
                                nc.tensor.matmul(
                                    vps[:], xr[:, xs_], wvt["r"][:, ws],
                                    start=(dc == 0), stop=False)
                                nc.tensor.matmul(
                                    vps[:], xi[:, xs_], wvt["i"][:, ws],
                                    start=False, stop=(dc == DC - 1))
                            nc.vector.tensor_copy(
                                va[:, kc * 2 * P:(kc + 1) * 2 * P], vps[:])
                        continue
                    wA = (wt[t + "a0"], wt[t + "a1"])
                    wB = (wt[t + "b0"], wt[t + "b1"])
                    pwide = sps.tile([P, WBLK], F32, tag="sps", name="projw")
                    psr = pwide[:, 0:TBLK]
                    psi = pwide[:, TBLK:WBLK]
                    for dc in range(DC):
                        ws = slice(dc * P, (dc + 1) * P)
                        xs_ = slice(dc * TBLK, (dc + 1) * TBLK)
                        nc.tensor.matmul(psr, wA[0][:, ws], xr[:, xs_],
                                         start=(dc == 0), stop=False)
                        nc.tensor.matmul(psi, wA[1][:, ws], xr[:, xs_],
                                         start=(dc == 0), stop=False)
                    for dc in range(DC):
                        ws = slice(dc * P, (dc + 1) * P)
                        xs_ = slice(dc * TBLK, (dc + 1) * TBLK)
                        nc.tensor.matmul(psr, wB[0][:, ws], xi[:, xs_],
                                         start=False, stop=(dc == DC - 1))
                        nc.tensor.matmul(psi, wB[1][:, ws], xi[:, xs_],
                                         start=False, stop=(dc == DC - 1))
                    hs = slice(half * TBLK, (half + 1) * TBLK)
                    if t == "q":
                        # psX = [q_r(h); q_i(h)] = Qcat directly
                        for h, psx in ((0, psr), (1, psi)):
                            nc.vector.tensor_copy(qcat[h][:, hs], psx)
                    else:
                        # psX = [k_r(h); -k_i(h)] = Kcat_r directly;
                        # Kcat_i = [k_i; k_r] via one negate + one copy
                        for h, psx in ((0, psr), (1, psi)):
                            nc.vector.tensor_copy(kcr[h][:, hs], psx)
                            nc.vector.tensor_scalar_mul(
                                kci[h][0:DH, hs], psx[DH:P, :], -1.0)
                            nc.vector.tensor_copy(kci[h][DH:P, hs],
                                                  psx[0:DH, :])

        def emit_attention(b, qcat, kcr, kci, va, o_stage):
            units = [(h, qb, kc)
                     for h in range(HPC) for qb in range(2)
                     for kc in range(KC)]
            swides = [None] * len(units)
            accs = {}

            def emit_scores(n):
                h, qb, kc = units[n]
                qs = slice(qb * TBLK, (qb + 1) * TBLK)
                ks = slice(kc * P, (kc + 1) * P)
                sw = sps.tile([P, WBLK], F32, tag="sps", name="scorew")
                nc.tensor.matmul(sw[:, 0:TBLK], kcr[h][:, ks],
                                 qcat[h][:, qs], start=True, stop=True)
                nc.tensor.matmul(sw[:, TBLK:WBLK], kci[h][:, ks],
                                 qcat[h][:, qs], start=True, stop=True)
                swides[n] = sw

            emit_scores(0)
            for n, (h, qb, kc) in enumerate(units):
                if n + 1 < len(units):
                    emit_scores(n + 1)
                uw = upool.tile([P, WBLK], BF16, tag="u", name="u")
                nc.scalar.activation(uw[:], swides[n][:], EXP)
                ur = uw[:, 0:TBLK]
                ui = uw[:, TBLK:WBLK]
                ks = slice(kc * P, (kc + 1) * P)
                first, last = kc == 0, kc == KC - 1
                if first:
                    zr = zps_pool.tile([P, TBLK], F32, tag="zr", name="zr")
                    zi = zps_pool.tile([P, TBLK], F32, tag="zi", name="zi")
                    oa = ops_pool.tile([P, TBLK], F32, tag="oa", name="oa")
                    ob = ops_pool.tile([P, TBLK], F32, tag="ob", name="ob")
                    accs[(h, qb)] = (zr, zi, oa, ob)
                else:
                    zr, zi, oa, ob = accs[(h, qb)]
                vsl = va[:, kc * 2 * P + h * P:kc * 2 * P + (h + 1) * P]
                nc.tensor.matmul(zr[:], ones[:], ur,
                                 start=first, stop=last)
                nc.tensor.matmul(zi[:], ones[:], ui,
                                 start=first, stop=last)
                nc.tensor.matmul(oa[:], vsl, ur,
                                 start=first, stop=last)
                nc.tensor.matmul(ob[:], vsl, ui,
                                 start=first, stop=last)
                if last:
                    # o_r = (v_r.T u_r)/Z_r - (v_i.T u_i)/Z_i : each AV term
                    # gets its OWN softmax denominator.  ob = va.T u_i =
                    # [v_r.T u_i; v_i.T u_i], so its halves are read
                    # swapped in the combine below.  Ordered so each
                    # accumulator bank frees as early as possible.
                    qs = slice(qb * TBLK, (qb + 1) * TBLK)
                    zinv_r = zpool.tile([P, TBLK], F32, tag="zinvr",
                                        name="zinv_r")
                    nc.vector.reciprocal_approx_fast(zinv_r[:], zr[:])
                    zinv_i = zpool.tile([P, TBLK], F32, tag="zinvi",
                                        name="zinv_i")
                    nc.vector.reciprocal_approx_fast(zinv_i[:], zi[:])
                    tmpa = tmppool.tile([P, TBLK], F32, tag="tmpa",
                                        name="tmpa")
                    nc.vector.tensor_mul(tmpa[:], oa[:], zinv_r[:])
                    # tmpb halves written SWAPPED (ob = [v_r.T u_i; v_i.T
                    # u_i]); psum+sbuf inputs are exempt from the DVE
                    # same-base-partition rule, sbuf+sbuf is not.
                    tmpb = tmppool.tile([P, TBLK], F32, tag="tmpb",
                                        name="tmpb")
                    nc.vector.tensor_mul(tmpb[0:DH, :], ob[DH:P, :],
                                         zinv_i[DH:P, :])
                    nc.vector.tensor_mul(tmpb[DH:P, :], ob[0:DH, :],
                                         zinv_i[0:DH, :])
                    dst = slice(DH * h, DH * (h + 1))
                    nc.vector.tensor_sub(o_stage["r"][dst, qs],
                                         tmpa[0:DH, :], tmpb[0:DH, :])
                    nc.vector.tensor_add(o_stage["i"][dst, qs],
                                         tmpa[DH:P, :], tmpb[DH:P, :])

        def emit_oproj(b, o_stage):
            for half in range(2):
                hs = slice(half * TBLK, (half + 1) * TBLK)
                gt = 2 * b + half
                powide = popool.tile([P, 2 * DC * TBLK], BF16, tag="pow",
                                     name="powide")
                for mc in range(DC):
                    ms = slice(mc * P, (mc + 1) * P)
                    ow2 = sps.tile([P, WBLK], F32, tag="sps", name="ojw")
                    pr = ow2[:, 0:TBLK]
                    pi = ow2[:, TBLK:WBLK]
                    nc.tensor.matmul(pr, wot["r"][:, ms],
                                     o_stage["r"][:, hs],
                                     start=True, stop=False)
                    nc.tensor.matmul(pr, wot["in"][:, ms],
                                     o_stage["i"][:, hs],
                                     start=False, stop=True)
                    nc.tensor.matmul(pi, wot["i"][:, ms],
                                     o_stage["r"][:, hs],
                                     start=True, stop=False)
                    nc.tensor.matmul(pi, wot["r"][:, ms],
                                     o_stage["i"][:, hs],
                                     start=False, stop=True)
                    # one wide copy: [pr | pi] -> powide cols 2*mc..2*mc+2
                    nc.vector.tensor_copy(
                        powide[:, 2 * mc * TBLK:(2 * mc + 2) * TBLK], ow2[:])
                # store on the gpsimd SWDGE queue: the sync queue must stay
                # dedicated to input prefetch (a store blocking the sync
                # FIFO head starves the next projection phase)
                nc.gpsimd.dma_start(po_ap[gt * P:(gt + 1) * P, :], powide[:])

        # ---- pipelined emission: one continuous tensor stream ----
        emit_xloads(0)
        stage = {}

        def new_stage(b):
            qcat = [qkpool.tile([P, S], BF16, tag=f"qcat{h}", name=f"qcat{h}")
                    for h in range(HPC)]
            kcr = [qkpool.tile([P, S], BF16, tag=f"kcr{h}", name=f"kcr{h}")
                   for h in range(HPC)]
            kci = [qkpool.tile([P, S], BF16, tag=f"kci{h}", name=f"kci{h}")
                   for h in range(HPC)]
            # va: [128 tokens-in-chunk, kc*256 + h*128 + [v_r(64)|v_i(64)]]
            va = vpool.tile([P, 2 * S], BF16, tag="va", name="va")
            o_stage = {p: opool.tile([P, S], BF16, tag=f"ost{p}",
                                     name=f"ost{p}")
                       for p in ("r", "i")}
            stage[b] = (qcat, kcr, kci, va, o_stage)

        new_stage(0)
        emit_proj(0, *stage[0][:4])
        for b in range(B):
            if b + 1 < B:
                emit_xloads(b + 1)
            emit_attention(b, *stage[b])
            if b + 1 < B:
                new_stage(b + 1)
                emit_proj(b + 1, *stage[b + 1][:4])
            emit_oproj(b, stage[b][4])
            del stage[b]

    nc.compile()
    return nc


def _w_sbuf_layout(w_t):
    """[D, 128] weight-transpose slice -> SBUF layout [128, dc*128+o]."""
    return np.ascontiguousarray(
        w_t.reshape(DC, P, P).transpose(1, 0, 2).reshape(P, D))


def _tile_x(xT, dtype):
    """[D, B*S] -> partition-major [NT*P, DC*TBLK] (row gt*P+p, col dc*TBLK+t)."""
    t = xT.reshape(DC, P, NT, TBLK).transpose(2, 1, 0, 3)
    return np.ascontiguousarray(t.reshape(NT * P, DC * TBLK)).astype(dtype)


def _prepare_in_maps(inputs):
    bf = ml_dtypes.bfloat16
    xs = {}
    for name, t in (("queries", "q"), ("keys", "k"), ("values", "v")):
        x = np.asarray(inputs[name], dtype=np.float32)  # [B,S,D,2]
        flat = x.reshape(B * S, D, 2)
        xs[t + "r"] = _tile_x(flat[:, :, 0].T, bf)
        xs[t + "i"] = _tile_x(flat[:, :, 1].T, bf)

    scale = np.float32(1.0 / np.sqrt(DH))
    in_maps = []
    for c in range(NCORES):
        rows = slice(P * c, P * (c + 1))
        m = {}
        for t in ("q", "k", "v"):
            for part in ("r", "i"):
                m[f"x{t}_{part}"] = xs[t + part]
        for t, wr_name, wi_name in (("q", "wq_r", "wq_i"),
                                    ("k", "wk_r", "wk_i")):
            s = scale if t == "q" else np.float32(1.0)
            wr = np.asarray(inputs[wr_name], dtype=np.float32)[rows] * s
            wi = np.asarray(inputs[wi_name], dtype=np.float32)[rows] * s
            for h in range(HPC):
                hr = slice(DH * h, DH * (h + 1))
                if t == "q":
                    wa = np.concatenate([wr[hr].T, wi[hr].T], axis=1)
                    wb = np.concatenate([-wi[hr].T, wr[hr].T], axis=1)
                else:
                    wa = np.concatenate([wr[hr].T, -wi[hr].T], axis=1)
                    wb = np.concatenate([-wi[hr].T, -wr[hr].T], axis=1)
                m[f"w{t}_a{h}"] = _w_sbuf_layout(wa).astype(bf)
                m[f"w{t}_b{h}"] = _w_sbuf_layout(wb).astype(bf)
        # V^T weights, moving operand: [1024 d, 2 heads * (v_r 64 | v_i 64)]
        # chunked to [128, dc*256 + c]
        wvr = np.asarray(inputs["wv_r"], dtype=np.float32)[rows]  # [128, 1024]
        wvi = np.asarray(inputs["wv_i"], dtype=np.float32)[rows]
        br = np.concatenate(
            [np.concatenate([wvr[DH * h:DH * (h + 1)].T,
                             wvi[DH * h:DH * (h + 1)].T], axis=1)
             for h in range(HPC)], axis=1)  # [1024, 256]
        bi = np.concatenate(
            [np.concatenate([-wvi[DH * h:DH * (h + 1)].T,
                             wvr[DH * h:DH * (h + 1)].T], axis=1)
             for h in range(HPC)], axis=1)
        m["wvt_r"] = np.ascontiguousarray(
            br.reshape(DC, P, 2 * P).transpose(1, 0, 2).reshape(
                P, DC * 2 * P)).astype(bf)
        m["wvt_i"] = np.ascontiguousarray(
            bi.reshape(DC, P, 2 * P).transpose(1, 0, 2).reshape(
                P, DC * 2 * P)).astype(bf)
        wo_r = np.asarray(inputs["wo_r"], dtype=np.float32)[:, rows]  # [D,128]
        wo_i = np.asarray(inputs["wo_i"], dtype=np.float32)[:, rows]
        m["wo_r"] = np.ascontiguousarray(wo_r.T).astype(bf)  # [128 d, 1024 m]
        m["wo_i"] = np.ascontiguousarray(wo_i.T).astype(bf)
        m["wo_in"] = np.ascontiguousarray(-wo_i.T).astype(bf)
        m["onesin"] = np.ones((P, P), dtype=bf)
        in_maps.append(m)
    return in_maps


LAST_RESULT = None


def _run(inputs, trace=False):
    global LAST_RESULT
    from concourse.bass_utils import run_bass_kernel_spmd
    if "nc" not in _CACHE:
        _CACHE["nc"] = _build()
    nc = _CACHE["nc"]
    in_maps = _prepare_in_maps(inputs)
    if trace:
        os.environ.pop("BASS_NEVER_TRACE", None)
    else:
        os.environ["BASS_NEVER_TRACE"] = "1"
    res = run_bass_kernel_spmd(nc, in_maps, core_ids=list(range(NCORES)),
                               trace=trace)
    LAST_RESULT = res
    # po rows gt*P+p, cols (2*mc+ri)*TBLK+tok
    acc = np.zeros((NT * P, 2 * DC * TBLK), np.float32)
    for c in range(NCORES):
        acc += res.results[c]["po"].astype(np.float32)

    # [NT, P, DC, 2, TBLK] -> ri, [D=DC*P? no: d=mc*P+p] , tokens
    t = acc.reshape(NT, P, DC, 2, TBLK)
    out = np.empty((B, S, D, 2), np.float32)
    for ri in range(2):
        # value at [gt, p, mc, ri, tok] = out_part[d=mc*128+p, gt*512+tok]
        comp = t[:, :, :, ri, :].transpose(2, 1, 0, 3).reshape(D, B * S)
        out[..., ri] = comp.T.reshape(B, S, D)
    return out


def kernel(**inputs):
    return _run(inputs, trace=False)



# revision 11
# speedup vs baseline: 1.3222x; 1.1095x over previous
"""ComplexMultiHeadAttention on 8 TRN2 NeuronCores (Bass/Tile) — fused stream.

Problem: B=4, S=1024, D_MODEL=1024, N_HEADS=16, D_HEAD=64, complex-valued
activations stored as a trailing dim of size 2 (real, imag).

    q = to_heads(complex_linear(queries, wq));  k, v likewise
    s_r + i*s_i = (q_r + i q_i)(k_r + i k_i)^T / sqrt(dh)
    a_r = softmax(s_r), a_i = softmax(s_i)      (independent softmaxes)
    o = complex_bmm(a, v);  out = complex_linear(concat_heads(o), wo)

Sharding: head-parallel. Core c owns heads {2c, 2c+1} = 128 contiguous dims
of the hidden axis. Weights row-sliced for QKV, wo column-sliced; the host
sums the 8 partial outputs — no on-device collectives.

Key design points (TRN2):
  - ONE fused tensor stream: attention(b) is interleaved, per key-chunk
    unit, with "filler" matmuls from oproj(b-1) and the q/k/v projections
    of (b+1).  Every engine's work is spread over the whole batch window,
    so no phase boundary ever idles the PE (which would also drop the
    DVFS p-state to half rate for ~5us).
  - V is projected directly in TRANSPOSED form: V^T = X^T W per 128-token
    block (X slice stationary, weights moving, 256-wide streams).  The
    value matrix lands token-major straight out of the PE — no DMA
    transposes, nothing on the scalar queue but the exps.
  - vb elimination: ob = va^T u_i and the epilogue reads its halves
    swapped (the complex cross terms only differ by that swap).
  - Z (softmax denominators): u chunks are accumulated with bf16 vector
    adds into u_acc per (head, q-half); ONE ones-matmul pair per group
    replaces 8 — Z tensor cols drop 8x and two PSUM banks are freed,
    which is exactly what lets scores/AV/aux all fit in 8 banks.
  - PSUM: scores 2x2 banks, AV wide 1x2 banks, aux (proj/oproj/Z) 2x1.
  - All matmuls bf16 (f32 PSUM accumulation); softmax over keys skips
    max-subtraction (scores are O(1) by construction).
"""

import os
import numpy as np
import ml_dtypes
from contextlib import ExitStack

import concourse.bass as bass
import concourse.tile as tile
from concourse import bacc, mybir

F32 = mybir.dt.float32
BF16 = mybir.dt.bfloat16
EXP = mybir.ActivationFunctionType.Exp

B, S, D, H, DH = 4, 1024, 1024, 16, 64
NCORES = 8
P = 128            # partitions / chunk size
TBLK = 512         # token block (matmul free dim)
WBLK = 2 * TBLK    # wide tile (2 psum banks)
DC = D // P        # 8 d-chunks
KC = S // P        # 8 key chunks per batch
HPC = H // NCORES  # 2 heads per core
NT = (B * S) // TBLK
NUNITS = HPC * 2 * KC  # 32 attention units per batch

_CACHE = {}


def _build():
    nc = bacc.Bacc("TRN2", target_bir_lowering=False, debug=False,
                   num_devices=NCORES)

    # partition-major tiled layout: row gt*128+p, col dc*512+tok
    x_ap = {}
    for t in ("q", "k", "v"):
        for part in ("r", "i"):
            x_ap[t + part] = nc.dram_tensor(
                f"x{t}_{part}", [NT * P, DC * TBLK],
                BF16, kind="ExternalInput").ap()
    w_ap = {}
    for t in ("q", "k"):
        for h in range(HPC):
            for suf in ("a", "b"):
                w_ap[f"{t}{suf}{h}"] = nc.dram_tensor(
                    f"w{t}_{suf}{h}", [P, D], BF16, kind="ExternalInput").ap()
    wvt_ap = {}
    for part in ("r", "i"):
        wvt_ap[part] = nc.dram_tensor(
            f"wvt_{part}", [P, DC * 2 * P], BF16, kind="ExternalInput").ap()
    wo_ap = {}
    for suf in ("r", "i", "in"):
        wo_ap[suf] = nc.dram_tensor(
            f"wo_{suf}", [P, D], BF16, kind="ExternalInput").ap()
    ones_ap = nc.dram_tensor("onesin", [P, P], BF16, kind="ExternalInput").ap()
    # output: row gt*128+p, col (2*mc+ri)*512+tok  (r/i interleaved per mc)
    po_ap = nc.dram_tensor("po", [NT * P, 2 * DC * TBLK], BF16,
                           kind="ExternalOutput").ap()

    with tile.TileContext(nc) as tc, ExitStack() as ctx:
        wpool = ctx.enter_context(tc.tile_pool(name="w", bufs=1))
        xpool = ctx.enter_context(tc.tile_pool(name="x", bufs=8))
        qkpool = ctx.enter_context(tc.tile_pool(name="qk", bufs=2))
        vpool = ctx.enter_context(tc.tile_pool(name="v", bufs=2))
        opool = ctx.enter_context(tc.tile_pool(name="ost", bufs=2))
        upool = ctx.enter_context(tc.tile_pool(name="u", bufs=3))
        uaccpool = ctx.enter_context(tc.tile_pool(name="uacc", bufs=2))
        zpool = ctx.enter_context(tc.tile_pool(name="z", bufs=2))
        tmppool = ctx.enter_context(tc.tile_pool(name="tmp", bufs=2))
        popool = ctx.enter_context(tc.tile_pool(name="po", bufs=2))
        # PSUM: scores 2x2 banks + AV wide 1x2 banks + aux 2x1 bank = 8
        sps = ctx.enter_context(tc.tile_pool(name="sp", bufs=2, space="PSUM"))
        avps = ctx.enter_context(tc.tile_pool(name="av", bufs=1,
                                              space="PSUM"))
        auxps = ctx.enter_context(tc.tile_pool(name="ax", bufs=2,
                                               space="PSUM"))

        # weights on the scalar HWDGE queue; V^T weights first (they gate
        # the very first projection matmuls), then q/k, wo/ones last.
        wvt = {}
        for part in ("r", "i"):
            wvt[part] = wpool.tile([P, DC * 2 * P], BF16, tag=f"wvt_{part}",
                                   name=f"wvt_{part}")
            nc.scalar.dma_start(wvt[part][:], wvt_ap[part][:])
        wt = {}
        for key, ap in list(w_ap.items()):
            wt[key] = wpool.tile([P, D], BF16, tag=f"w_{key}", name=f"w_{key}")
            nc.scalar.dma_start(wt[key][:], ap[:])
        wot = {}
        for suf, ap in wo_ap.items():
            wot[suf] = wpool.tile([P, D], BF16, tag=f"wo_{suf}",
                                  name=f"wo_{suf}")
            nc.scalar.dma_start(wot[suf][:], ap[:])
        ones = wpool.tile([P, P], BF16, tag="ones", name="ones")
        nc.scalar.dma_start(ones[:], ones_ap[:])

        xtiles = {}

        def emit_xloads(b):
            # v first (projection order is v,q,k); both parts of a half
            # together (the V^T projection consumes r+i of one half first)
            for t in ("v", "q", "k"):
                for half in range(2):
                    for part in ("r", "i"):
                        gt = 2 * b + half
                        xt = xpool.tile([P, DC * TBLK], BF16, tag="xt",
                                        name="xt")
                        nc.sync.dma_start(
                            xt[:], x_ap[t + part][gt * P:(gt + 1) * P, :])
                        xtiles[(b, t, part, half)] = xt

        def proj_gen(b, qcat, kcr, kci, va):
            """Projection of batch b as a stream of tensor quanta.

            Yields the emitted tensor-column count after each quantum.
            v first (V^T form), then q, then k; trailing vector ops of a
            psum group are emitted with its final quantum.
            """
            for half in range(2):
                xr = xtiles.pop((b, "v", "r", half))
                xi = xtiles.pop((b, "v", "i", half))
                for tb in range(4):
                    kc = half * 4 + tb
                    vps = auxps.tile([P, 2 * P], F32, tag="aux", name="vps")
                    for dcg in range(2):
                        for dc in range(dcg * 4, dcg * 4 + 4):
                            xs_ = slice(dc * TBLK + tb * P,
                                        dc * TBLK + (tb + 1) * P)
                            ws = slice(dc * 2 * P, (dc + 1) * 2 * P)
                            nc.tensor.matmul(
                                vps[:], xr[:, xs_], wvt["r"][:, ws],
                                start=(dc == 0), stop=False)
                            nc.tensor.matmul(
                                vps[:], xi[:, xs_], wvt["i"][:, ws],
                                start=False, stop=(dc == DC - 1))
                        if dcg == 1:
                            nc.vector.tensor_copy(
                                va[:, kc * 2 * P:(kc + 1) * 2 * P], vps[:])
                        yield 2048
            for t in ("q", "k"):
                for half in range(2):
                    xr = xtiles.pop((b, t, "r", half))
                    xi = xtiles.pop((b, t, "i", half))
                    hs = slice(half * TBLK, (half + 1) * TBLK)
                    for hh in range(2):
                        ps = auxps.tile([P, TBLK], F32, tag="aux",
                                        name="qkps")
                        wA = wt[f"{t}a{hh}"]
                        wB = wt[f"{t}b{hh}"]
                        for dcg in range(2):
                            for dc in range(dcg * 4, dcg * 4 + 4):
                                ws = slice(dc * P, (dc + 1) * P)
                                xs_ = slice(dc * TBLK, (dc + 1) * TBLK)
                                nc.tensor.matmul(
                                    ps[:], wA[:, ws], xr[:, xs_],
                                    start=(dc == 0), stop=False)
                                nc.tensor.matmul(
                                    ps[:], wB[:, ws], xi[:, xs_],
                                    start=False, stop=(dc == DC - 1))
                            if dcg == 1:
                                if t == "q":
                                    nc.vector.tensor_copy(qcat[hh][:, hs],
                                                          ps[:])
                                else:
                                    nc.vector.tensor_copy(kcr[hh][:, hs],
                                                          ps[:])
                                    nc.vector.tensor_scalar_mul(
                                        kci[hh][0:DH, hs], ps[DH:P, :], -1.0)
                                    nc.vector.tensor_copy(kci[hh][DH:P, hs],
                                                          ps[0:DH, :])
                            yield 4096

        def oproj_gen(b, o_stage):
            """O-projection of batch b as a stream of tensor quanta."""
            for half in range(2):
                gt = 2 * b + half
                hs = slice(half * TBLK, (half + 1) * TBLK)
                powide = popool.tile([P, 2 * DC * TBLK], BF16, tag="pow",
                                     name="powide")
                for mc in range(DC):
                    ms = slice(mc * P, (mc + 1) * P)
                    for ri in range(2):
                        ps = auxps.tile([P, TBLK], F32, tag="aux",
                                        name="ops")
                        if ri == 0:
                            pairs = ((wot["r"], o_stage["r"]),
                                     (wot["in"], o_stage["i"]))
                        else:
                            pairs = ((wot["i"], o_stage["r"]),
                                     (wot["r"], o_stage["i"]))
                        nc.tensor.matmul(ps[:], pairs[0][0][:, ms],
                                         pairs[0][1][:, hs],
                                         start=True, stop=False)
                        nc.tensor.matmul(ps[:], pairs[1][0][:, ms],
                                         pairs[1][1][:, hs],
                                         start=False, stop=True)
                        c0 = (2 * mc + ri) * TBLK
                        nc.vector.tensor_copy(powide[:, c0:c0 + TBLK], ps[:])
                        yield 1024
                # store on the gpsimd SWDGE queue (sync queue stays
                # dedicated to input prefetch)
                nc.gpsimd.dma_start(po_ap[gt * P:(gt + 1) * P, :], powide[:])

        def drain(gen):
            for _ in gen:
                pass

        def emit_window(b, qcat, kcr, kci, va, o_stage, filler):
            """Attention units of batch b with filler interleaved."""
            units = [(h, qb, kc)
                     for h in range(HPC) for qb in range(2)
                     for kc in range(KC)]
            total_fill = (32768 if b >= 1 else 0) + \
                         (98304 if b + 1 < B else 0)
            per_unit = (total_fill + NUNITS - 1) // NUNITS
            swides = [None] * len(units)
            accs = {}
            budget = 0

            def emit_scores(n):
                h, qb, kc = units[n]
                qs = slice(qb * TBLK, (qb + 1) * TBLK)
                ks = slice(kc * P, (kc + 1) * P)
                sw = sps.tile([P, WBLK], F32, tag="sps", name="scorew")
                nc.tensor.matmul(sw[:, 0:TBLK], kcr[h][:, ks],
                                 qcat[h][:, qs], start=True, stop=True)
                nc.tensor.matmul(sw[:, TBLK:WBLK], kci[h][:, ks],
                                 qcat[h][:, qs], start=True, stop=True)
                swides[n] = sw

            emit_scores(0)
            for n, (h, qb, kc) in enumerate(units):
                if n + 1 < len(units):
                    emit_scores(n + 1)
                first, last = kc == 0, kc == KC - 1
                if first:
                    uacc = uaccpool.tile([P, WBLK], BF16, tag="uacc",
                                         name="uacc")
                    avw = avps.tile([P, WBLK], F32, tag="av", name="avw")
                    accs[(h, qb)] = (uacc, avw)
                    u = uacc
                    nc.scalar.activation(uacc[:], swides[n][:], EXP)
                else:
                    uacc, avw = accs[(h, qb)]
                    u = upool.tile([P, WBLK], BF16, tag="u", name="u")
                    nc.scalar.activation(u[:], swides[n][:], EXP)
                    nc.vector.tensor_add(uacc[:], uacc[:], u[:])
                swides[n] = None
                # filler
                budget += per_unit
                while budget > 0 and filler:
                    try:
                        budget -= next(filler[0])
                    except StopIteration:
                        filler.pop(0)
                # AV
                vsl = va[:, kc * 2 * P + h * P:kc * 2 * P + (h + 1) * P]
                nc.tensor.matmul(avw[:, 0:TBLK], vsl, u[:, 0:TBLK],
                                 start=first, stop=last)
                nc.tensor.matmul(avw[:, TBLK:WBLK], vsl, u[:, TBLK:WBLK],
                                 start=first, stop=last)
                if last:
                    # Z = ones^T u_acc (one matmul pair per (h,qb)), then
                    # o_r = (v_r.T u_r)/Z_r - (v_i.T u_i)/Z_i etc.
                    # ob (= avw cols TBLK:) is va^T u_i, halves swapped in
                    # the combine; psum+sbuf DVE inputs are exempt from
                    # the same-base-partition rule.
                    del accs[(h, qb)]
                    qs = slice(qb * TBLK, (qb + 1) * TBLK)
                    zps_r = auxps.tile([P, TBLK], F32, tag="aux", name="zpr")
                    nc.tensor.matmul(zps_r[:], ones[:], uacc[:, 0:TBLK],
                                     start=True, stop=True)
                    zps_i = auxps.tile([P, TBLK], F32, tag="aux", name="zpi")
                    nc.tensor.matmul(zps_i[:], ones[:], uacc[:, TBLK:WBLK],
                                     start=True, stop=True)
                    zinv = zpool.tile([P, WBLK], F32, tag="zinv",
                                      name="zinv")
                    nc.vector.reciprocal_approx_fast(zinv[:, 0:TBLK],
                                                     zps_r[:])
                    nc.vector.reciprocal_approx_fast(zinv[:, TBLK:WBLK],
                                                     zps_i[:])
                    tmpa = tmppool.tile([P, TBLK], F32, tag="tmpa",
                                        name="tmpa")
                    nc.vector.tensor_mul(tmpa[:], avw[:, 0:TBLK],
                                         zinv[:, 0:TBLK])
                    tmpb = tmppool.tile([P, TBLK], F32, tag="tmpb",
                                        name="tmpb")
                    nc.vector.tensor_mul(tmpb[0:DH, :],
                                         avw[DH:P, TBLK:WBLK],
                                         zinv[DH:P, TBLK:WBLK])
                    nc.vector.tensor_mul(tmpb[DH:P, :],
                                         avw[0:DH, TBLK:WBLK],
                                         zinv[0:DH, TBLK:WBLK])
                    dst = slice(DH * h, DH * (h + 1))
                    nc.vector.tensor_sub(o_stage["r"][dst, qs],
                                         tmpa[0:DH, :], tmpb[0:DH, :])
                    nc.vector.tensor_add(o_stage["i"][dst, qs],
                                         tmpa[DH:P, :], tmpb[DH:P, :])
            # drain leftover filler
            for g in filler:
                drain(g)

        # ---- pipelined emission: one continuous tensor stream ----
        stage = {}

        def new_stage(b):
            qcat = [qkpool.tile([P, S], BF16, tag=f"qcat{h}", name=f"qcat{h}")
                    for h in range(HPC)]
            kcr = [qkpool.tile([P, S], BF16, tag=f"kcr{h}", name=f"kcr{h}")
                   for h in range(HPC)]
            kci = [qkpool.tile([P, S], BF16, tag=f"kci{h}", name=f"kci{h}")
                   for h in range(HPC)]
            # va: [128 tokens-in-chunk, kc*256 + h*128 + [v_r(64)|v_i(64)]]
            va = vpool.tile([P, 2 * S], BF16, tag="va", name="va")
            o_stage = {p: opool.tile([P, S], BF16, tag=f"ost{p}",
                                     name=f"ost{p}")
                       for p in ("r", "i")}
            stage[b] = (qcat, kcr, kci, va, o_stage)

        emit_xloads(0)
        new_stage(0)
        drain(proj_gen(0, *stage[0][:4]))
        for b in range(B):
            if b + 1 < B:
                emit_xloads(b + 1)
                new_stage(b + 1)
            filler = []
            if b >= 1:
                filler.append(oproj_gen(b - 1, stage[b - 1][4]))
            if b + 1 < B:
                filler.append(proj_gen(b + 1, *stage[b + 1][:4]))
            emit_window(b, *stage[b], filler)
            if b >= 1:
                del stage[b - 1]
        drain(oproj_gen(B - 1, stage[B - 1][4]))

    nc.compile()
    return nc


def _w_sbuf_layout(w_t):
    """[D, 128] weight-transpose slice -> SBUF layout [128, dc*128+o]."""
    return np.ascontiguousarray(
        w_t.reshape(DC, P, P).transpose(1, 0, 2).reshape(P, D))


def _tile_x(xT, dtype):
    """[D, B*S] -> partition-major [NT*P, DC*TBLK] (row gt*P+p, col dc*TBLK+t)."""
    t = xT.reshape(DC, P, NT, TBLK).transpose(2, 1, 0, 3)
    return np.ascontiguousarray(t.reshape(NT * P, DC * TBLK)).astype(dtype)


def _prepare_in_maps(inputs):
    bf = ml_dtypes.bfloat16
    xs = {}
    for name, t in (("queries", "q"), ("keys", "k"), ("values", "v")):
        x = np.asarray(inputs[name], dtype=np.float32)  # [B,S,D,2]
        flat = x.reshape(B * S, D, 2)
        xs[t + "r"] = _tile_x(flat[:, :, 0].T, bf)
        xs[t + "i"] = _tile_x(flat[:, :, 1].T, bf)

    scale = np.float32(1.0 / np.sqrt(DH))
    in_maps = []
    for c in range(NCORES):
        rows = slice(P * c, P * (c + 1))
        m = {}
        for t in ("q", "k", "v"):
            for part in ("r", "i"):
                m[f"x{t}_{part}"] = xs[t + part]
        for t, wr_name, wi_name in (("q", "wq_r", "wq_i"),
                                    ("k", "wk_r", "wk_i")):
            s = scale if t == "q" else np.float32(1.0)
            wr = np.asarray(inputs[wr_name], dtype=np.float32)[rows] * s
            wi = np.asarray(inputs[wi_name], dtype=np.float32)[rows] * s
            for h in range(HPC):
                hr = slice(DH * h, DH * (h + 1))
                if t == "q":
                    wa = np.concatenate([wr[hr].T, wi[hr].T], axis=1)
                    wb = np.concatenate([-wi[hr].T, wr[hr].T], axis=1)
                else:
                    wa = np.concatenate([wr[hr].T, -wi[hr].T], axis=1)
                    wb = np.concatenate([-wi[hr].T, -wr[hr].T], axis=1)
                m[f"w{t}_a{h}"] = _w_sbuf_layout(wa).astype(bf)
                m[f"w{t}_b{h}"] = _w_sbuf_layout(wb).astype(bf)
        # V^T weights, moving operand: [1024 d, 2 heads * (v_r 64 | v_i 64)]
        # chunked to [128, dc*256 + c]
        wvr = np.asarray(inputs["wv_r"], dtype=np.float32)[rows]  # [128,1024]
        wvi = np.asarray(inputs["wv_i"], dtype=np.float32)[rows]
        br = np.concatenate(
            [np.concatenate([wvr[DH * h:DH * (h + 1)].T,
                             wvi[DH * h:DH * (h + 1)].T], axis=1)
             for h in range(HPC)], axis=1)  # [1024, 256]
        bi = np.concatenate(
            [np.concatenate([-wvi[DH * h:DH * (h + 1)].T,
                             wvr[DH * h:DH * (h + 1)].T], axis=1)
             for h in range(HPC)], axis=1)
        m["wvt_r"] = np.ascontiguousarray(
            br.reshape(DC, P, 2 * P).transpose(1, 0, 2).reshape(
                P, DC * 2 * P)).astype(bf)
        m["wvt_i"] = np.ascontiguousarray(
            bi.reshape(DC, P, 2 * P).transpose(1, 0, 2).reshape(
                P, DC * 2 * P)).astype(bf)
        wo_r = np.asarray(inputs["wo_r"], dtype=np.float32)[:, rows]  # [D,128]
        wo_i = np.asarray(inputs["wo_i"], dtype=np.float32)[:, rows]
        m["wo_r"] = np.ascontiguousarray(wo_r.T).astype(bf)  # [128 d, 1024 m]
        m["wo_i"] = np.ascontiguousarray(wo_i.T).astype(bf)
        m["wo_in"] = np.ascontiguousarray(-wo_i.T).astype(bf)
        m["onesin"] = np.ones((P, P), dtype=bf)
        in_maps.append(m)
    return in_maps


LAST_RESULT = None


def _run(inputs, trace=False):
    global LAST_RESULT
    from concourse.bass_utils import run_bass_kernel_spmd
    if "nc" not in _CACHE:
        _CACHE["nc"] = _build()
    nc = _CACHE["nc"]
    in_maps = _prepare_in_maps(inputs)
    if trace:
        os.environ.pop("BASS_NEVER_TRACE", None)
    else:
        os.environ["BASS_NEVER_TRACE"] = "1"
    res = run_bass_kernel_spmd(nc, in_maps, core_ids=list(range(NCORES)),
                               trace=trace)
    LAST_RESULT = res
    # po rows gt*P+p, cols (2*mc+ri)*TBLK+tok
    acc = np.zeros((NT * P, 2 * DC * TBLK), np.float32)
    for c in range(NCORES):
        acc += res.results[c]["po"].astype(np.float32)

    t = acc.reshape(NT, P, DC, 2, TBLK)
    out = np.empty((B, S, D, 2), np.float32)
    for ri in range(2):
        # value at [gt, p, mc, ri, tok] = out_part[d=mc*128+p, gt*512+tok]
        comp = t[:, :, :, ri, :].transpose(2, 1, 0, 3).reshape(D, B * S)
        out[..., ri] = comp.T.reshape(B, S, D)
    return out


def kernel(**inputs):
    return _run(inputs, trace=False)


# revision 19
# speedup vs baseline: 1.3612x; 1.0295x over previous
"""ComplexMultiHeadAttention on 8 TRN2 NeuronCores (Bass/Tile) — fused stream.

Problem: B=4, S=1024, D_MODEL=1024, N_HEADS=16, D_HEAD=64, complex-valued
activations stored as a trailing dim of size 2 (real, imag).

    q = to_heads(complex_linear(queries, wq));  k, v likewise
    s_r + i*s_i = (q_r + i q_i)(k_r + i k_i)^T / sqrt(dh)
    a_r = softmax(s_r), a_i = softmax(s_i)      (independent softmaxes)
    o = complex_bmm(a, v);  out = complex_linear(concat_heads(o), wo)

Sharding: head-parallel. Core c owns heads {2c, 2c+1} = 128 contiguous dims
of the hidden axis. Weights row-sliced for QKV, wo column-sliced; the host
sums the 8 partial outputs — no on-device collectives.

Key design points (TRN2):
  - ONE fused tensor stream: attention(b) is interleaved, per key-chunk
    unit, with "filler" matmuls from oproj(b-1) and the q/k/v projections
    of (b+1).  Every engine's work is spread over the whole batch window,
    so no phase boundary ever idles the PE (which would also drop the
    DVFS p-state to half rate for ~5us).
  - V is projected directly in TRANSPOSED form: V^T = X^T W per 128-token
    block (X slice stationary, weights moving, 256-wide streams).  The
    value matrix lands token-major straight out of the PE — no DMA
    transposes, nothing on the scalar queue but the exps.
  - vb elimination: ob = va^T u_i and the epilogue reads its halves
    swapped (the complex cross terms only differ by that swap).
  - Z (softmax denominators): u chunks are accumulated with bf16 vector
    adds into u_acc per (head, q-half); ONE ones-matmul pair per group
    replaces 8 — Z tensor cols drop 8x and two PSUM banks are freed,
    which is exactly what lets scores/AV/aux all fit in 8 banks.
  - PSUM: scores 2x2 banks, AV wide 1x2 banks, aux (proj/oproj/Z) 2x1.
  - All matmuls bf16 (f32 PSUM accumulation); softmax over keys skips
    max-subtraction (scores are O(1) by construction).
"""

import os
import numpy as np
import ml_dtypes
from contextlib import ExitStack

import concourse.bass as bass
import concourse.tile as tile
from concourse import bacc, mybir

F32 = mybir.dt.float32
BF16 = mybir.dt.bfloat16
EXP = mybir.ActivationFunctionType.Exp

B, S, D, H, DH = 4, 1024, 1024, 16, 64
NCORES = 8
P = 128            # partitions / chunk size
TBLK = 512         # token block (matmul free dim)
WBLK = 2 * TBLK    # wide tile (2 psum banks)
DC = D // P        # 8 d-chunks
KC = S // P        # 8 key chunks per batch
HPC = H // NCORES  # 2 heads per core
NT = (B * S) // TBLK
NUNITS = HPC * 2 * KC  # 32 attention units per batch

_CACHE = {}


def _build():
    nc = bacc.Bacc("TRN2", target_bir_lowering=False, debug=False,
                   num_devices=NCORES)

    # partition-major tiled layout: row gt*128+p, col dc*512+tok
    x_ap = {}
    for t in ("q", "k", "v"):
        for part in ("r", "i"):
            x_ap[t + part] = nc.dram_tensor(
                f"x{t}_{part}", [NT * P, DC * TBLK],
                BF16, kind="ExternalInput").ap()
    w_ap = {}
    for t in ("q", "k"):
        for h in range(HPC):
            for suf in ("a", "b"):
                w_ap[f"{t}{suf}{h}"] = nc.dram_tensor(
                    f"w{t}_{suf}{h}", [P, D], BF16, kind="ExternalInput").ap()
    wvt_ap = {}
    for part in ("r", "i"):
        wvt_ap[part] = nc.dram_tensor(
            f"wvt_{part}", [P, DC * 2 * P], BF16, kind="ExternalInput").ap()
    wo_ap = {}
    for suf in ("r", "i", "in"):
        wo_ap[suf] = nc.dram_tensor(
            f"wo_{suf}", [P, D], BF16, kind="ExternalInput").ap()
    ones_ap = nc.dram_tensor("onesin", [P, P], BF16, kind="ExternalInput").ap()
    # output: row gt*128+p, col (2*mc+ri)*512+tok  (r/i interleaved per mc)
    po_ap = nc.dram_tensor("po", [NT * P, 2 * DC * TBLK], BF16,
                           kind="ExternalOutput").ap()

    with tile.TileContext(nc) as tc, ExitStack() as ctx:
        wpool = ctx.enter_context(tc.tile_pool(name="w", bufs=1))
        xpool = ctx.enter_context(tc.tile_pool(name="x", bufs=8))
        qkpool = ctx.enter_context(tc.tile_pool(name="qk", bufs=2))
        vpool = ctx.enter_context(tc.tile_pool(name="v", bufs=2))
        opool = ctx.enter_context(tc.tile_pool(name="ost", bufs=2))
        upool = ctx.enter_context(tc.tile_pool(name="u", bufs=3))
        uaccpool = ctx.enter_context(tc.tile_pool(name="uacc", bufs=2))
        zpool = ctx.enter_context(tc.tile_pool(name="z", bufs=2))
        tmppool = ctx.enter_context(tc.tile_pool(name="tmp", bufs=2))
        popool = ctx.enter_context(tc.tile_pool(name="po", bufs=2))
        # PSUM: scores 2x2 banks + AV wide 1x2 banks + aux 2x1 bank = 8
        sps = ctx.enter_context(tc.tile_pool(name="sp", bufs=2, space="PSUM"))
        avps = ctx.enter_context(tc.tile_pool(name="av", bufs=1,
                                              space="PSUM"))
        auxps = ctx.enter_context(tc.tile_pool(name="ax", bufs=2,
                                               space="PSUM"))

        # Startup DMAs all on the sync queue in CONSUMPTION-priority order
        # (HBM bandwidth is the startup limiter; arrival order must match
        # the proj(0) consumption order): wvt, xv(0), wq, xq(0), wk,
        # xk(0), wo/ones.  Later batches' x loads queue behind.
        wt = {}
        wot = {}
        wvt = {}
        xtiles = {}

        def load_w(dst, key, ap, pfx):
            dst[key] = wpool.tile(list(ap.shape), BF16, tag=f"{pfx}_{key}",
                                  name=f"{pfx}_{key}")
            nc.sync.dma_start(dst[key][:], ap[:])

        def emit_xloads(b, t):
            for half in range(2):
                for part in ("r", "i"):
                    gt = 2 * b + half
                    xt = xpool.tile([P, DC * TBLK], BF16, tag="xt",
                                    name="xt")
                    nc.sync.dma_start(
                        xt[:], x_ap[t + part][gt * P:(gt + 1) * P, :])
                    xtiles[(b, t, part, half)] = xt

        for part in ("r", "i"):
            load_w(wvt, part, wvt_ap[part], "wvt")
        emit_xloads(0, "v")
        for h in range(HPC):
            for suf in ("a", "b"):
                load_w(wt, f"q{suf}{h}", w_ap[f"q{suf}{h}"], "w")
        emit_xloads(0, "q")
        for h in range(HPC):
            for suf in ("a", "b"):
                load_w(wt, f"k{suf}{h}", w_ap[f"k{suf}{h}"], "w")
        emit_xloads(0, "k")
        for suf, ap in wo_ap.items():
            load_w(wot, suf, ap, "wo")
        ones = wpool.tile([P, P], BF16, tag="ones", name="ones")
        nc.sync.dma_start(ones[:], ones_ap[:])

        def proj_gen(b, qcat, kcr, kci, va):
            """Projection of batch b as a stream of tensor quanta.

            Yields the emitted tensor-column count after each quantum.
            v first (V^T form), then q, then k; trailing vector ops of a
            psum group are emitted with its final quantum.
            """
            for half in range(2):
                xr = xtiles.pop((b, "v", "r", half))
                xi = xtiles.pop((b, "v", "i", half))
                for tb in range(4):
                    kc = half * 4 + tb
                    vps = auxps.tile([P, 2 * P], F32, tag="aux", name="vps")
                    for dcg in range(2):
                        for dc in range(dcg * 4, dcg * 4 + 4):
                            xs_ = slice(dc * TBLK + tb * P,
                                        dc * TBLK + (tb + 1) * P)
                            ws = slice(dc * 2 * P, (dc + 1) * 2 * P)
                            nc.tensor.matmul(
                                vps[:], xr[:, xs_], wvt["r"][:, ws],
                                start=(dc == 0), stop=False)
                            nc.tensor.matmul(
                                vps[:], xi[:, xs_], wvt["i"][:, ws],
                                start=False, stop=(dc == DC - 1))
                        if dcg == 1:
                            nc.vector.tensor_copy(
                                va[:, kc * 2 * P:(kc + 1) * 2 * P], vps[:])
                        yield 2048
            for t in ("q", "k"):
                for half in range(2):
                    xr = xtiles.pop((b, t, "r", half))
                    xi = xtiles.pop((b, t, "i", half))
                    hs = slice(half * TBLK, (half + 1) * TBLK)
                    for hh in range(2):
                        ps = auxps.tile([P, TBLK], F32, tag="aux",
                                        name="qkps")
                        wA = wt[f"{t}a{hh}"]
                        wB = wt[f"{t}b{hh}"]
                        for dcg in range(2):
                            for dc in range(dcg * 4, dcg * 4 + 4):
                                ws = slice(dc * P, (dc + 1) * P)
                                xs_ = slice(dc * TBLK, (dc + 1) * TBLK)
                                nc.tensor.matmul(
                                    ps[:], wA[:, ws], xr[:, xs_],
                                    start=(dc == 0), stop=False)
                                nc.tensor.matmul(
                                    ps[:], wB[:, ws], xi[:, xs_],
                                    start=False, stop=(dc == DC - 1))
                            if dcg == 1:
                                if t == "q":
                                    nc.vector.tensor_copy(qcat[hh][:, hs],
                                                          ps[:])
                                else:
                                    nc.vector.tensor_copy(kcr[hh][:, hs],
                                                          ps[:])
                                    nc.vector.tensor_scalar_mul(
                                        kci[hh][0:DH, hs], ps[DH:P, :], -1.0)
                                    nc.vector.tensor_copy(kci[hh][DH:P, hs],
                                                          ps[0:DH, :])
                            yield 4096

        def oproj_gen(b, o_stage, halves=(0, 1)):
            """O-projection of batch b as a stream of tensor quanta.

            po is stored in quarter chunks (gpsimd SWDGE) so the final
            store's DMA tail is short and earlier chunks overlap compute.
            """
            for half in halves:
                gt = 2 * b + half
                hs = slice(half * TBLK, (half + 1) * TBLK)
                powide = popool.tile([P, 2 * DC * TBLK], BF16, tag="pow",
                                     name="powide")
                for mc in range(DC):
                    ms = slice(mc * P, (mc + 1) * P)
                    for ri in range(2):
                        ps = auxps.tile([P, TBLK], F32, tag="aux",
                                        name="ops")
                        if ri == 0:
                            pairs = ((wot["r"], o_stage["r"]),
                                     (wot["in"], o_stage["i"]))
                        else:
                            pairs = ((wot["i"], o_stage["r"]),
                                     (wot["r"], o_stage["i"]))
                        nc.tensor.matmul(ps[:], pairs[0][0][:, ms],
                                         pairs[0][1][:, hs],
                                         start=True, stop=False)
                        nc.tensor.matmul(ps[:], pairs[1][0][:, ms],
                                         pairs[1][1][:, hs],
                                         start=False, stop=True)
                        c0 = (2 * mc + ri) * TBLK
                        nc.vector.tensor_copy(powide[:, c0:c0 + TBLK], ps[:])
                        yield 1024
                    if mc % 2 == 1:
                        cs = slice((2 * mc - 2) * TBLK, (2 * mc + 2) * TBLK)
                        nc.gpsimd.dma_start(po_ap[gt * P:(gt + 1) * P, cs],
                                            powide[:, cs])

        def drain(gen):
            for _ in gen:
                pass

        def emit_window(b, qcat, kcr, kci, va, o_stage, filler,
                        mid_filler=None):
            """Attention units of batch b with filler interleaved.

            qb-outer unit order: both heads' q-half epilogues complete by
            mid-window, so `mid_filler` (last batch's own half-0 oproj)
            can be injected after unit 15.
            """
            units = [(h, qb, kc)
                     for qb in range(2) for h in range(HPC)
                     for kc in range(KC)]
            total_fill = (32768 if b >= 1 else 0) + \
                         (98304 if b + 1 < B else 0)
            per_unit = (total_fill + NUNITS - 1) // NUNITS
            swides = [None] * len(units)
            accs = {}
            budget = 0

            def emit_scores(n):
                h, qb, kc = units[n]
                qs = slice(qb * TBLK, (qb + 1) * TBLK)
                ks = slice(kc * P, (kc + 1) * P)
                sw = sps.tile([P, WBLK], F32, tag="sps", name="scorew")
                nc.tensor.matmul(sw[:, 0:TBLK], kcr[h][:, ks],
                                 qcat[h][:, qs], start=True, stop=True)
                nc.tensor.matmul(sw[:, TBLK:WBLK], kci[h][:, ks],
                                 qcat[h][:, qs], start=True, stop=True)
                swides[n] = sw

            emit_scores(0)
            for n, (h, qb, kc) in enumerate(units):
                if n + 1 < len(units):
                    emit_scores(n + 1)
                if n == NUNITS // 2 and mid_filler is not None:
                    filler.append(mid_filler)
                    per_unit += 1024
                first, last = kc == 0, kc == KC - 1
                if first:
                    uacc = uaccpool.tile([P, WBLK], BF16, tag="uacc",
                                         name="uacc")
                    avw = avps.tile([P, WBLK], F32, tag="av", name="avw")
                    accs[(h, qb)] = (uacc, avw)
                    u = uacc
                    nc.scalar.activation(uacc[:], swides[n][:], EXP)
                else:
                    uacc, avw = accs[(h, qb)]
                    u = upool.tile([P, WBLK], BF16, tag="u", name="u")
                    nc.scalar.activation(u[:], swides[n][:], EXP)
                    nc.vector.tensor_add(uacc[:], uacc[:], u[:])
                swides[n] = None
                # filler; kc==0 units get a bonus pop so the avw-reuse
                # boundary (previous group's epilogue muls) stays hidden
                budget += per_unit + (1024 if first else 0)
                while budget > 0 and filler:
                    try:
                        budget -= next(filler[0])
                    except StopIteration:
                        filler.pop(0)
                # AV
                vsl = va[:, kc * 2 * P + h * P:kc * 2 * P + (h + 1) * P]
                nc.tensor.matmul(avw[:, 0:TBLK], vsl, u[:, 0:TBLK],
                                 start=first, stop=last)
                nc.tensor.matmul(avw[:, TBLK:WBLK], vsl, u[:, TBLK:WBLK],
                                 start=first, stop=last)
                if last:
                    # Z = ones^T u_acc (one matmul pair per (h,qb)), then
                    # o_r = (v_r.T u_r)/Z_r - (v_i.T u_i)/Z_i etc.
                    # ob (= avw cols TBLK:) is va^T u_i, halves swapped in
                    # the combine; psum+sbuf DVE inputs are exempt from
                    # the same-base-partition rule.
                    del accs[(h, qb)]
                    qs = slice(qb * TBLK, (qb + 1) * TBLK)
                    zps_r = auxps.tile([P, TBLK], F32, tag="aux", name="zpr")
                    nc.tensor.matmul(zps_r[:], ones[:], uacc[:, 0:TBLK],
                                     start=True, stop=True)
                    zps_i = auxps.tile([P, TBLK], F32, tag="aux", name="zpi")
                    nc.tensor.matmul(zps_i[:], ones[:], uacc[:, TBLK:WBLK],
                                     start=True, stop=True)
                    zinv = zpool.tile([P, WBLK], F32, tag="zinv",
                                      name="zinv")
                    nc.vector.reciprocal_approx_fast(zinv[:, 0:TBLK],
                                                     zps_r[:])
                    nc.vector.reciprocal_approx_fast(zinv[:, TBLK:WBLK],
                                                     zps_i[:])
                    tmpa = tmppool.tile([P, TBLK], F32, tag="tmpa",
                                        name="tmpa")
                    nc.vector.tensor_mul(tmpa[:], avw[:, 0:TBLK],
                                         zinv[:, 0:TBLK])
                    tmpb = tmppool.tile([P, TBLK], F32, tag="tmpb",
                                        name="tmpb")
                    nc.vector.tensor_mul(tmpb[0:DH, :],
                                         avw[DH:P, TBLK:WBLK],
                                         zinv[DH:P, TBLK:WBLK])
                    nc.vector.tensor_mul(tmpb[DH:P, :],
                                         avw[0:DH, TBLK:WBLK],
                                         zinv[0:DH, TBLK:WBLK])
                    dst = slice(DH * h, DH * (h + 1))
                    nc.vector.tensor_sub(o_stage["r"][dst, qs],
                                         tmpa[0:DH, :], tmpb[0:DH, :])
                    nc.vector.tensor_add(o_stage["i"][dst, qs],
                                         tmpa[DH:P, :], tmpb[DH:P, :])
            # drain leftover filler
            for g in filler:
                drain(g)

        # ---- pipelined emission: one continuous tensor stream ----
        stage = {}

        def new_stage(b):
            qcat = [qkpool.tile([P, S], BF16, tag=f"qcat{h}", name=f"qcat{h}")
                    for h in range(HPC)]
            kcr = [qkpool.tile([P, S], BF16, tag=f"kcr{h}", name=f"kcr{h}")
                   for h in range(HPC)]
            kci = [qkpool.tile([P, S], BF16, tag=f"kci{h}", name=f"kci{h}")
                   for h in range(HPC)]
            # va: [128 tokens-in-chunk, kc*256 + h*128 + [v_r(64)|v_i(64)]]
            va = vpool.tile([P, 2 * S], BF16, tag="va", name="va")
            o_stage = {p: opool.tile([P, S], BF16, tag=f"ost{p}",
                                     name=f"ost{p}")
                       for p in ("r", "i")}
            stage[b] = (qcat, kcr, kci, va, o_stage)

        new_stage(0)
        drain(proj_gen(0, *stage[0][:4]))
        for b in range(B):
            if b + 1 < B:
                for t in ("v", "q", "k"):
                    emit_xloads(b + 1, t)
                new_stage(b + 1)
            filler = []
            if b >= 1:
                filler.append(oproj_gen(b - 1, stage[b - 1][4]))
            if b + 1 < B:
                filler.append(proj_gen(b + 1, *stage[b + 1][:4]))
            # last window: inject this batch's own half-0 oproj once both
            # q-half-0 epilogues are in (qb-outer order, after unit 15)
            mid = (oproj_gen(b, stage[b][4], halves=(0,))
                   if b == B - 1 else None)
            emit_window(b, *stage[b], filler, mid_filler=mid)
            if b >= 1:
                del stage[b - 1]
        drain(oproj_gen(B - 1, stage[B - 1][4], halves=(1,)))

    nc.compile()
    return nc


def _w_sbuf_layout(w_t):
    """[D, 128] weight-transpose slice -> SBUF layout [128, dc*128+o]."""
    return np.ascontiguousarray(
        w_t.reshape(DC, P, P).transpose(1, 0, 2).reshape(P, D))


def _tile_x(xT, dtype):
    """[D, B*S] -> partition-major [NT*P, DC*TBLK] (row gt*P+p, col dc*TBLK+t)."""
    t = xT.reshape(DC, P, NT, TBLK).transpose(2, 1, 0, 3)
    return np.ascontiguousarray(t.reshape(NT * P, DC * TBLK)).astype(dtype)


def _prepare_in_maps(inputs):
    bf = ml_dtypes.bfloat16
    xs = {}
    for name, t in (("queries", "q"), ("keys", "k"), ("values", "v")):
        x = np.asarray(inputs[name], dtype=np.float32)  # [B,S,D,2]
        flat = x.reshape(B * S, D, 2)
        xs[t + "r"] = _tile_x(flat[:, :, 0].T, bf)
        xs[t + "i"] = _tile_x(flat[:, :, 1].T, bf)

    scale = np.float32(1.0 / np.sqrt(DH))
    in_maps = []
    for c in range(NCORES):
        rows = slice(P * c, P * (c + 1))
        m = {}
        for t in ("q", "k", "v"):
            for part in ("r", "i"):
                m[f"x{t}_{part}"] = xs[t + part]
        for t, wr_name, wi_name in (("q", "wq_r", "wq_i"),
                                    ("k", "wk_r", "wk_i")):
            s = scale if t == "q" else np.float32(1.0)
            wr = np.asarray(inputs[wr_name], dtype=np.float32)[rows] * s
            wi = np.asarray(inputs[wi_name], dtype=np.float32)[rows] * s
            for h in range(HPC):
                hr = slice(DH * h, DH * (h + 1))
                if t == "q":
                    wa = np.concatenate([wr[hr].T, wi[hr].T], axis=1)
                    wb = np.concatenate([-wi[hr].T, wr[hr].T], axis=1)
                else:
                    wa = np.concatenate([wr[hr].T, -wi[hr].T], axis=1)
                    wb = np.concatenate([-wi[hr].T, -wr[hr].T], axis=1)
                m[f"w{t}_a{h}"] = _w_sbuf_layout(wa).astype(bf)
                m[f"w{t}_b{h}"] = _w_sbuf_layout(wb).astype(bf)
        # V^T weights, moving operand: [1024 d, 2 heads * (v_r 64 | v_i 64)]
        # chunked to [128, dc*256 + c]
        wvr = np.asarray(inputs["wv_r"], dtype=np.float32)[rows]  # [128,1024]
        wvi = np.asarray(inputs["wv_i"], dtype=np.float32)[rows]
        br = np.concatenate(
            [np.concatenate([wvr[DH * h:DH * (h + 1)].T,
                             wvi[DH * h:DH * (h + 1)].T], axis=1)
             for h in range(HPC)], axis=1)  # [1024, 256]
        bi = np.concatenate(
            [np.concatenate([-wvi[DH * h:DH * (h + 1)].T,
                             wvr[DH * h:DH * (h + 1)].T], axis=1)
             for h in range(HPC)], axis=1)
        m["wvt_r"] = np.ascontiguousarray(
            br.reshape(DC, P, 2 * P).transpose(1, 0, 2).reshape(
                P, DC * 2 * P)).astype(bf)
        m["wvt_i"] = np.ascontiguousarray(
            bi.reshape(DC, P, 2 * P).transpose(1, 0, 2).reshape(
                P, DC * 2 * P)).astype(bf)
        wo_r = np.asarray(inputs["wo_r"], dtype=np.float32)[:, rows]  # [D,128]
        wo_i = np.asarray(inputs["wo_i"], dtype=np.float32)[:, rows]
        m["wo_r"] = np.ascontiguousarray(wo_r.T).astype(bf)  # [128 d, 1024 m]
        m["wo_i"] = np.ascontiguousarray(wo_i.T).astype(bf)
        m["wo_in"] = np.ascontiguousarray(-wo_i.T).astype(bf)
        m["onesin"] = np.ones((P, P), dtype=bf)
        in_maps.append(m)
    return in_maps


LAST_RESULT = None


def _run(inputs, trace=False):
    global LAST_RESULT
    from concourse.bass_utils import run_bass_kernel_spmd
    if "nc" not in _CACHE:
        _CACHE["nc"] = _build()
    nc = _CACHE["nc"]
    in_maps = _prepare_in_maps(inputs)
    if trace:
        os.environ.pop("BASS_NEVER_TRACE", None)
    else:
        os.environ["BASS_NEVER_TRACE"] = "1"
    res = run_bass_kernel_spmd(nc, in_maps, core_ids=list(range(NCORES)),
                               trace=trace)
    LAST_RESULT = res
    # po rows gt*P+p, cols (2*mc+ri)*TBLK+tok
    acc = np.zeros((NT * P, 2 * DC * TBLK), np.float32)
    for c in range(NCORES):
        acc += res.results[c]["po"].astype(np.float32)

    t = acc.reshape(NT, P, DC, 2, TBLK)
    out = np.empty((B, S, D, 2), np.float32)
    for ri in range(2):
        # value at [gt, p, mc, ri, tok] = out_part[d=mc*128+p, gt*512+tok]
        comp = t[:, :, :, ri, :].transpose(2, 1, 0, 3).reshape(D, B * S)
        out[..., ri] = comp.T.reshape(B, S, D)
    return out


def kernel(**inputs):
    return _run(inputs, trace=False)
